# revision 10
# baseline (speedup 1.0000x reference)
"""Trainium2 Bass kernel for nn_DiffNet (gnn_message_passing).

The reference's per-element "edge MLP" over the meta stack
(vi, W, vj) -> two 1x1 convs -> weighted sum over the input dim is
linear in its 3 channels, so it collapses algebraically.  With
g = conv1_w.T @ conv2_w[0]  (3 scalars), hb = conv1_b@conv2_w[0]+conv2_b[0],
z = vi @ W.T (no bias), s1[b] = sum_i vi[b,i], s2[b] = sum_i vi[b,i]^2:

    out[b,o] = relu(z+b)[b,o] * (1 + scale*g2*s1[b])
             + scale*(g0*s2[b] + g1*z[b,o] + hb*s1[b])

so the whole network is 3 small matmuls + elementwise, and the problem
is memory-bound on the fc weights.

v3 (vs the 32-36us f32r baseline, which was bound by a single HWDGE
queue streaming 3.4 MB of fp32 at ~158 GB/s with double-pass LOW_HIGH
matmuls):
  * everything on the matmul dataflow is fp16 -> 1.6 MB of wire
    traffic and single-pass PE matmuls (tolerance is 2e-2; measured
    baseline err 1.1e-3, fp16 adds ~1e-3).
  * DMAs are split half/half between the qSync and qScalar HWDGE
    rings so the 16 shared SDMA engines are fed from two descriptor
    rings; x is its own small tensor loaded FIRST on both rings (in
    v2 it rode the slow SWDGE ring and landed at 14us, idling every
    engine); the biases+eye ride the tail of the w1 stream.
  * tiny constants (ones column for the K-dir sums, the alpha/beta
    coefficient matrices) are memset on-chip instead of DMAed.
  * the PE HAM clock gate defaults to 1.2 GHz and only ramps to
    2.4 GHz after ~3.4us of sustained activity (v2's z matmuls all
    ran at cold-clock rate, 427ns per 512-wide chunk).  A run of
    dummy matmuls on memset scratch keeps the PE busy from ~1us
    while the weights stream in, so the real z matmuls run warm.

Distribution (8 cores, no collectives): fc1/fc2 replicated, fc3
sharded over its output dim (32 cols/core); full batch B=32 on every
core; host concatenates the 8 [32,32] output shards.

On-core layout: activations live transposed [feature(partitions),
batch] in 128-row chunks; weights are passed pre-transposed [in, out].
Matmuls put the tiny activation tile stationary and stream the weight
chunk [128, 512] as the moving operand.  The z output lands
[batch, out] in fp32 PSUM; a PE transpose brings each 128-col chunk
back to [out, batch] where relu-bias, the k1*z term and the per-batch
alpha/beta scalars (broadcast across partitions via a rank-1 ones
matmul) are applied with a few wide DVE/ACT ops.
"""

import sys

if "/opt/trn_rl_repo" not in sys.path:
    sys.path.insert(0, "/opt/trn_rl_repo")

import numpy as np


def _install_ntff_hook_shim():
    """This image's antenv lacks ``axon_hooks``; bass_utils hard-imports it
    when tracing under axon.  Provide the module and register the ctypes
    NTFF hook from trn_agent_boot so ``trace=True`` yields exec_time_ns."""
    import types

    if "antenv.axon_hooks" in sys.modules:
        return
    try:
        import antenv

        mod = types.ModuleType("antenv.axon_hooks")
        _h = [None]
        mod.set_axon_ntff_profile_hook = lambda hook: _h.__setitem__(0, hook)
        mod.get_axon_ntff_profile_hook = lambda: _h[0]
        sys.modules["antenv.axon_hooks"] = mod
        antenv.axon_hooks = mod
        from trn_agent_boot.trn_boot import _ntff_profile_via_ctypes

        mod.set_axon_ntff_profile_hook(
            _ntff_profile_via_ctypes("/opt/axon/libaxon_pjrt.so")
        )
    except Exception:
        pass


_install_ntff_hook_shim()

N_CORES = 8
B = 32
I1, O1, O2, O3 = 1024, 512, 512, 256
O3L = O3 // N_CORES  # fc3 output cols per core
RATE = 0.1

# aux columns (fp16) appended to the w1 tensor: b12 | b3 | eye
AUX_B12 = 8 * O1  # [128, 8]  fc1_b / fc2_b as 4+4 cols
AUX_B3 = AUX_B12 + 8  # [0:O3L, 1]  fc3_b shard
AUX_EYE = AUX_B3 + 1  # [0:B, B]  identity (fp16, cast to f32 on-chip)
W1W = AUX_EYE + B  # w1 DRAM tensor total cols
W2W = 4 * O2 + 4 * O3L  # w2 DRAM tensor cols: w2 | w3

_CACHE = {}
LAST_RESULTS = None  # BassKernelResults of the most recent run (for test.py)


def _build(k0, k1, k2, kb):
    import concourse.bacc as bacc
    import concourse.mybir as mybir
    import concourse.tile as tile
    import concourse.bass as bass

    f32 = mybir.dt.float32
    f16 = mybir.dt.float16
    AF = mybir.ActivationFunctionType
    ALU = mybir.AluOpType

    from concourse.tile_rust import add_dep_helper

    nc = bacc.Bacc(
        "TRN2", target_bir_lowering=False, debug=False, num_devices=N_CORES
    )

    xt = nc.declare_dram_parameter("xt", [128, 8 * B], f16, isOutput=False)
    w1 = nc.declare_dram_parameter("w1t", [128, W1W], f16, isOutput=False)
    w2 = nc.declare_dram_parameter("w2t", [128, W2W], f16, isOutput=False)
    out_d = nc.declare_dram_parameter("out", [O3L, B], f32, isOutput=True)

    with tile.TileContext(nc) as tc:
        with (
            tc.tile_pool(name="wts", bufs=1) as wp,
            tc.tile_pool(name="act", bufs=1) as ap,
            tc.tile_pool(name="ps", bufs=1, space=bass.MemorySpace.PSUM) as pp,
        ):
            tx = wp.tile([128, 8 * B], f16, tag="x")
            tw1 = wp.tile([128, W1W], f16, tag="w1")
            tw2 = wp.tile([128, W2W], f16, tag="w2")
            tw3 = tw2[:, 4 * O2 : 4 * O2 + 4 * O3L]

            # on-chip constants (no DMA): ones col for K-dir sums, the
            # alpha/beta coefficient matrices, f32 cast of the biases
            t1k = wp.tile([128, 1], f16, tag="ones")
            tka = wp.tile([96, 128], f16, tag="ka")
            tkb = wp.tile([96, 128], f16, tag="kb")
            tb = wp.tile([128, 9], f32, tag="bias")  # b12 | b3
            teye = tw1[0:B, AUX_EYE : AUX_EYE + B]  # fp16 identity
            tscr = wp.tile([128, O1], f16, tag="scr")  # PE warm-up fodder

            # -- DMAs, split between the two HWDGE rings.  x whole on
            # qSync (512B lines; splitting would halve the line size,
            # not the time).  w1 in 4 chunks, 2 per ring, so z1 can
            # accumulate chunks in arrival order; biases+eye ride the
            # last 41 columns of the w1 stream's scalar half; w3 rides
            # the tail of w2's scalar half.
            nc.sync.dma_start(tx[:], xt[:])
            nc.sync.dma_start(tw1[:, 0 : 2 * O1], w1[:, 0 : 2 * O1])
            nc.scalar.dma_start(tw1[:, 4 * O1 : 6 * O1], w1[:, 4 * O1 : 6 * O1])
            nc.sync.dma_start(tw1[:, 2 * O1 : 4 * O1], w1[:, 2 * O1 : 4 * O1])
            nc.scalar.dma_start(tw1[:, 6 * O1 : W1W], w1[:, 6 * O1 : W1W])
            nc.sync.dma_start(tw2[:, 0 : 2 * O2], w2[:, 0 : 2 * O2])
            nc.scalar.dma_start(tw2[:, 2 * O2 : W2W], w2[:, 2 * O2 : W2W])

            # warm-up fodder + ones col first: the PE warm-up only
            # depends on these two memsets
            nc.vector.memset(tscr[:], 0.0)
            nc.vector.memset(t1k[:], 1.0)
            nc.vector.memset(tka[:], 0.0)
            nc.vector.memset(tkb[:], 0.0)
            # alpha = k2*s1 + 1 ; beta = kb*s1 + k0*s2
            # (s_sb rows: 0 = s1, 32 = s2, 64 = ones)
            nc.vector.memset(tka[0:1, :], k2)
            nc.vector.memset(tka[64:65, :], 1.0)
            nc.vector.memset(tkb[0:1, :], kb)
            nc.vector.memset(tkb[32:33, :], k0)
            # f32 cast of the fp16-shipped biases on DVE (an ACT copy
            # would block the Scalar queue's later s_sb copies behind
            # the w1 DMA this reads from)
            nc.vector.tensor_scalar_add(tb[:], tw1[:, AUX_B12 : AUX_B12 + 9], 0.0)
            tb12 = tb[:, 0:8]
            tb3 = tb[0:O3L, 8:9]

            # PE warm-up: the HAM clock gate only ramps 1.2 -> 2.4 GHz
            # after ~3.4us of sustained PE activity.  These dummy
            # matmuls depend only on the first two memsets, so they run
            # while the weights stream in; the real z matmuls start warm.
            # (warm_ps shares the "z" PSUM tag, so z1 orders after.)
            warm_ps = pp.tile([1, O1], f32, tag="z")
            for _ in range(7):
                nc.tensor.matmul(
                    warm_ps[:], t1k[:], tscr[:], start=True, stop=True
                )

            def ordered(dependent, dependency, why):
                if dependent is not None and dependency is not None:
                    add_dep_helper(
                        dependent.ins, dependency.ins, sync=False, reason=why
                    )

            def stats_ab(a_tile, n_c, n_oc, tag, after_mm=None):
                """a_tile [128, n_c*B] f16; -> (alpha, beta, bcast).
                alpha/beta [128, n_oc*B] f32: rows all equal, the same
                [128,B] per-batch scalars replicated n_oc times so the
                tail can apply them in one full-width DVE op."""
                asq = ap.tile([128, n_c * B], f16, tag=tag + "sq")
                nc.vector.tensor_tensor(asq[:], a_tile, a_tile, ALU.mult)
                s1_ps = pp.tile([1, B], f32, tag="s1")
                s2_ps = pp.tile([1, B], f32, tag="s2")
                mm1 = None
                for c in range(n_c):
                    mm = nc.tensor.matmul(
                        s1_ps[:],
                        t1k[:],
                        a_tile[:, c * B : (c + 1) * B],
                        start=(c == 0),
                        stop=(c == n_c - 1),
                    )
                    mm1 = mm1 or mm
                for c in range(n_c):
                    nc.tensor.matmul(
                        s2_ps[:],
                        t1k[:],
                        asq[:, c * B : (c + 1) * B],
                        start=(c == 0),
                        stop=(c == n_c - 1),
                    )
                ordered(mm1, after_mm, "stats after this layer's z matmuls")
                # engine writes must start at partition 0/32/64 -> spread
                # (s1, s2, 1) over those rows; memset first so junk
                # partitions are finite (their K coefficients are 0) and
                # row 64 is the ones row
                s_sb = ap.tile([96, B], f16, tag=tag + "row")
                nc.vector.memset(s_sb[:], 1.0)
                nc.scalar.copy(s_sb[0:1, :], s1_ps[:])
                nc.scalar.copy(s_sb[32:33, :], s2_ps[:])
                ab_ps = pp.tile([128, 2 * n_oc * B], f32, tag="ab")
                bcast = None
                for oc in range(n_oc):
                    nc.tensor.matmul(
                        ab_ps[:, oc * B : (oc + 1) * B],
                        tka[:], s_sb[:], start=True, stop=True,
                    )
                    bcast = nc.tensor.matmul(
                        ab_ps[:, (n_oc + oc) * B : (n_oc + oc + 1) * B],
                        tkb[:], s_sb[:], start=True, stop=True,
                    )
                ab_sb = ap.tile([128, 2 * n_oc * B], f32, tag=tag + "sb")
                nc.scalar.copy(ab_sb[:], ab_ps[:])
                return (
                    ab_sb[:, 0 : n_oc * B],
                    ab_sb[:, n_oc * B : 2 * n_oc * B],
                    bcast,
                )

            def z_mms(a_tile, w_tile, ics, ow, after=None):
                """z_ps [B, ow] = a.T @ w; `ics` gives the accumulation
                order (matched to the chunks' DMA arrival order)."""
                z_ps = pp.tile([B, ow], f32, tag="z")
                last = None
                for j, ic in enumerate(ics):
                    mm = nc.tensor.matmul(
                        z_ps[:],
                        a_tile[:, ic * B : (ic + 1) * B],
                        w_tile[:, ic * ow : (ic + 1) * ow],
                        start=(j == 0),
                        stop=(j == len(ics) - 1),
                    )
                    if j == 0:
                        ordered(mm, after, "z matmuls after stats bcast")
                    last = mm
                return z_ps, last

            def tail(z_ps, n_oc, ow, bias_col, alpha, beta, out_view, li,
                     after=None):
                """transpose z back to [out, batch] (fp16 single-pass);
                relu+bias per chunk; combine with alpha/beta full-width;
                writes out_view [np_out, n_oc*B]."""
                np_out = min(ow, 128)
                z_sb = ap.tile([B, ow], f16, tag=f"zsb{li}")
                nc.scalar.copy(z_sb[:], z_ps[:])
                zt_ps = pp.tile([np_out, n_oc * B], f16, tag="zt")
                for oc in range(n_oc):
                    tr = nc.tensor.transpose(
                        zt_ps[:, oc * B : (oc + 1) * B],
                        z_sb[:, oc * 128 : oc * 128 + np_out],
                        teye,
                    )
                    if oc == 0:
                        ordered(tr, after, "transposes after stats bcast")
                vjt = ap.tile([np_out, n_oc * B], f32, tag=f"vj{li}")
                t_sb = ap.tile([np_out, n_oc * B], f32, tag=f"t{li}")
                for oc in range(n_oc):
                    bsl = slice(oc * B, (oc + 1) * B)
                    # relu(z + bias): alternate ACT / DVE so neither engine
                    # paces the per-oc pipeline
                    if oc % 2 == 0:
                        nc.scalar.activation(
                            vjt[:, bsl], zt_ps[:, bsl], AF.Relu,
                            bias=bias_col(oc), scale=1.0,
                        )
                    else:
                        nc.vector.tensor_scalar(
                            vjt[:, bsl], zt_ps[:, bsl], bias_col(oc), 0.0,
                            ALU.add, ALU.max,
                        )
                # t = k1*z + beta ; out = vj*alpha + t  (full-width)
                nc.vector.scalar_tensor_tensor(
                    t_sb[:], zt_ps[:], k1, beta[0:np_out, :], ALU.mult, ALU.add
                )
                nc.vector.tensor_tensor(
                    vjt[:], vjt[:], alpha[0:np_out, :], ALU.mult
                )
                nc.vector.tensor_tensor(out_view[:], vjt[:], t_sb[:], ALU.add)

            # ---- forward chain: stats1 fills the PE while fc1 streams in;
            # later layers run stats between their z matmuls and transposes.
            # z accumulation orders match the queue arrival order of the
            # weight chunks (sync: x, ic01, ic23, w2ic01 / scalar: ic45,
            # ic67, w2ic23+w3).
            al1, be1, bc1 = stats_ab(tx, 8, 4, "ab1")
            z1, z1l = z_mms(tx, tw1, [4, 5, 0, 1, 6, 7, 2, 3], O1, after=bc1)
            a2 = ap.tile([128, 4 * B], f16, tag="a2")
            tail(z1, 4, O1, lambda oc: tb12[:, oc : oc + 1], al1, be1, a2[:], 1)

            z2, z2l = z_mms(a2[:], tw2, [2, 3, 0, 1], O2)
            al2, be2, bc2 = stats_ab(a2[:], 4, 4, "ab2", after_mm=z2l)
            a3 = ap.tile([128, 4 * B], f16, tag="a3")
            tail(z2, 4, O2, lambda oc: tb12[:, 4 + oc : 5 + oc], al2, be2,
                 a3[:], 2, after=bc2)

            z3, z3l = z_mms(a3[:], tw3, [0, 1, 2, 3], O3L)
            al3, be3, bc3 = stats_ab(a3[:], 4, 1, "ab3", after_mm=z3l)
            out_sb = ap.tile([O3L, B], f32, tag="o3")
            tail(z3, 1, O3L, lambda oc: tb3, al3, be3, out_sb[:], 3, after=bc3)

            nc.sync.dma_start(out_d[:], out_sb[:], single_packet=True)

    nc.compile()
    return nc


def kernel(**inputs):
    from concourse.bass_utils import run_bass_kernel_spmd

    x = np.ascontiguousarray(np.asarray(inputs["x"], dtype=np.float32))
    fc1_w = np.asarray(inputs["fc1_w"], dtype=np.float32)
    fc1_b = np.asarray(inputs["fc1_b"], dtype=np.float32)
    fc2_w = np.asarray(inputs["fc2_w"], dtype=np.float32)
    fc2_b = np.asarray(inputs["fc2_b"], dtype=np.float32)
    fc3_w = np.asarray(inputs["fc3_w"], dtype=np.float32)
    fc3_b = np.asarray(inputs["fc3_b"], dtype=np.float32)
    c1w = np.asarray(inputs["conv1_w"], dtype=np.float32)
    c1b = np.asarray(inputs["conv1_b"], dtype=np.float32)
    c2w = np.asarray(inputs["conv2_w"], dtype=np.float32)
    c2b = np.asarray(inputs["conv2_b"], dtype=np.float32)
    bn = float(np.asarray(inputs["batch_num"]).astype(np.float64))

    scale = np.float32(RATE) / np.float32(bn)
    g = (c1w.T @ c2w[0]).astype(np.float32)  # [3]
    hb = np.float32(c1b @ c2w[0] + c2b[0])
    k0 = float(scale * g[0])
    k1 = float(scale * g[1])
    k2 = float(scale * g[2])
    kb = float(scale * hb)

    key = (k0, k1, k2, kb)
    if key not in _CACHE:
        _CACHE[key] = _build(*key)
    nc = _CACHE[key]

    def pack(m, n_c, width):  # [n_c*128, width] -> [128, n_c*width]
        return np.ascontiguousarray(
            m.reshape(n_c, 128, width).transpose(1, 0, 2).reshape(128, n_c * width)
        )

    xt_h = pack(x.T, 8, B).astype(np.float16)
    w1_h = np.zeros((128, W1W), dtype=np.float16)
    w1_h[:, 0 : 8 * O1] = pack(fc1_w.T, 8, O1)
    w1_h[:, AUX_B12 : AUX_B12 + 4] = fc1_b.reshape(4, 128).T
    w1_h[:, AUX_B12 + 4 : AUX_B12 + 8] = fc2_b.reshape(4, 128).T
    w1_h[0:B, AUX_EYE : AUX_EYE + B] = np.eye(B, dtype=np.float16)
    w2_base = pack(fc2_w.T, 4, O2).astype(np.float16)

    in_maps = []
    for c in range(N_CORES):
        w1c = w1_h.copy()
        w1c[0:O3L, AUX_B3] = fc3_b[c * O3L : (c + 1) * O3L]
        w2c = np.zeros((128, W2W), dtype=np.float16)
        w2c[:, 0 : 4 * O2] = w2_base
        w2c[:, 4 * O2 :] = pack(
            fc3_w[c * O3L : (c + 1) * O3L].T, 4, O3L
        ).astype(np.float16)
        in_maps.append(dict(xt=xt_h, w1t=w1c, w2t=w2c))

    res = run_bass_kernel_spmd(nc, in_maps, list(range(N_CORES)))
    global LAST_RESULTS
    LAST_RESULTS = res
    return np.ascontiguousarray(
        np.concatenate([res.results[c]["out"].T for c in range(N_CORES)], axis=1)
    ).astype(np.float32)


if __name__ == "__main__":
    rng = np.random.default_rng(0)

    def lin(fo, fi):
        bound = 1.0 / np.sqrt(fi)
        return (
            rng.uniform(-bound, bound, (fo, fi)).astype(np.float32),
            rng.uniform(-bound, bound, (fo,)).astype(np.float32),
        )

    fc1_w, fc1_b = lin(512, 1024)
    fc2_w, fc2_b = lin(512, 512)
    fc3_w, fc3_b = lin(256, 512)
    c1w, c1b = lin(8, 3)
    c2w, c2b = lin(1, 8)
    ins = dict(
        x=rng.standard_normal((32, 1024)).astype(np.float32),
        fc1_w=fc1_w, fc1_b=fc1_b, fc2_w=fc2_w, fc2_b=fc2_b,
        fc3_w=fc3_w, fc3_b=fc3_b,
        conv1_w=c1w, conv1_b=c1b, conv2_w=c2w, conv2_b=c2b,
        batch_num=10,
    )
    out = kernel(**ins)
    print("kernel out", out.shape, out.dtype, float(np.abs(out).max()))
